# revision 18
# baseline (speedup 1.0000x reference)
"""DistMult decoder kernel for Trainium2 (8 NeuronCores, SPMD).

Computes rec = (inputs * relation) @ inputs.T for inputs [8192, 512] f32,
relation [512] f32, output [8192, 8192] f32.

Strategy: the output is symmetric (rec[m,n] = sum_k r_k x_mk x_nk), so only
~half the 512x512 blocks are computed on device; the mirror happens during
host-side assembly.  Work is balanced across 8 cores with a rotation trick
that keeps the program SPMD-uniform: core t gets X row-rotated by 2t*512 and
owns global row-blocks (2t, 2t+1), each computing its forward column window
(9 blocks) — 18 block-matmuls per core over a shared 10-column-block xt
window (see the SLOTS comment).  The four slots that are diagonal or
antipodal-duplicate blocks compute only their 128-strip lower triangle
(saving 8.3% of PE rows); the host mirrors diagonals and combines the two
transposed antipodal halves from cores t and t+4 (see TRI).  Matmuls run in
bf16 with fp32 PSUM accumulation; the stationary operand is mostly derived
on device (on the ACT engine) from the resident xt data and the relation
vector — except its first 128-column m-subtile, which the host ships
pre-computed so the first matmul can start ~1.3us in, right behind the
first xt quarter-chunk.  Outputs are staged and stored as fp16 and upcast
on host.  The host pre-tiles xt into the exact SBUF layout so all device
DMAs are contiguous; j0/j1 load as 128 KB quarter-chunks (fine-grained head
pacing), later blocks as 256 KB halves.  Stores are batched at group
granularity with a single-group final store, and the last (triangle) slot
emits its m-subtiles widest-first so the kernel tail is a w=128 group:
copy + 128 KB store + drain instead of the full-slot flush.
"""

import numpy as np
import ml_dtypes

import concourse.bass as bass
import concourse.mybir as mybir
import concourse.tile as tile
from concourse.bass_utils import run_bass_kernel_spmd
from concourse.vector_clock import ScopedClock


# When True, the next TileContext exit emits only the drain chain (no
# all-engine barrier / semaphore clears).  Safe only for the final context
# of the program: the SP drain chain waits on every semaphore, so SP ends
# last and NEFF completion still implies all work (incl. DMA) is done.
_SKIP_TAIL_BARRIER = False


def _split_drain_and_barrier(self, tick_clock, wait_clock):
    """Replacement for TileContext._drain_and_barrier that splits the tail
    drain's semaphore waits across multiple single-wait Drain instructions.
    The walrus build in this environment rejects instructions with more than
    a few sync waits ("Too many sync wait commands"), and the stock tail
    drain waits on every semaphore the kernel used."""
    nc = self.nc
    drain_inst = nc.sync.drain()
    wait_clock.add_sem_waits(
        drain_inst.ins, ScopedClock({None: tick_clock.global_clock})
    )
    si = drain_inst.ins.sync_info
    if si is not None and len(si.on_wait) > 1:
        waits = list(si.on_wait)
        updates = list(si.on_update)
        if _SKIP_TAIL_BARRIER:
            # Final context: the store DMAs (HWDGE) are the dependency
            # leaves -- every matmul/copy/load completion is transitively
            # implied by them (stores wait on copies, copies on matmuls,
            # matmuls on loads; engines retire in order).  Waiting only on
            # the store queues shortens the serial drain chain.
            hw = [w for w in waits if (w.ant_name or "").startswith("DMAHW")]
            if hw:
                waits = hw
        drain_inst.ins.sync_info = mybir.SyncInfo(on_wait=waits[:1], on_update=[])
        for i, w in enumerate(waits[1:]):
            last = i == len(waits) - 2
            d = nc.sync.drain()
            d.ins.sync_info = mybir.SyncInfo(
                on_wait=[w], on_update=updates if last else []
            )
        if len(waits) == 1 and updates:
            d = nc.sync.drain()
            d.ins.sync_info = mybir.SyncInfo(on_wait=[], on_update=updates)

    assert self.sems is not None
    popped = nc._tile_sem_poison_stack.pop()
    assert popped is self._sem_poison
    if _SKIP_TAIL_BARRIER:
        return
    nc.all_engine_barrier()
    nc.clear_and_free_semaphores(list(self.sems.allocated().values()))
    nc.all_engine_barrier()


tile.TileContext._drain_and_barrier = _split_drain_and_barrier

N = 8192            # rows of inputs
D = 512             # feature dim (contraction)
B = 512             # output block size
NB = N // B         # 16 blocks per side
C = 8               # cores
P = 128
KSUB = D // P       # 4 k-subtiles
MSUB = B // P       # 4 m-subtiles per block

# Antipodal-pair ownership: core t owns global row-blocks (t, t+8) -- BOTH
# blocks of one antipodal pair.  Row-block r computes its forward window of
# columns (r, r+1 .. r+7) mod 16 plus, for the h=0 row only, the antipodal
# column r+8.  This covers every unordered block pair exactly once (forward
# distances 1..7 from one side; distance-8 pairs {t, t+8} are wholly owned
# by core t and computed once, full-width, from the h=0 side), so per-core
# PE work hits the theoretical floor: 15 full + 2 diagonal-triangle slots =
# 133120 PE rows.  The cost is a full 16-column-block xt window (8 MB,
# 64 KB/partition -- fits SBUF comfortably).
#
# (m_block_local, col_local) per output slot.  m_block_local 0 -> local
# rows [0, 512) (global row-block t); 1 -> local rows [8*512, 9*512)
# (global t+8).  Local col j corresponds to global col-block (t + j).
# h=0 slots: j = 0 (diag triangle), 1..8; h=1 slots: j = 9..15, then the
# diag triangle (1, 8) LAST -- its widest-first m-subtiles make the final
# computed group a w=128 strip, shrinking the copy+store tail.
SLOTS = (
    [(0, j) for j in range(9)]
    + [(1, j) for j in range(9, 16)]
    + [(1, 8)]
)
NSLOT = len(SLOTS)  # 17
NBX = 16            # xt column-blocks resident per core

# Triangle slots: the two diagonal blocks (t,t) and (t+8,t+8) are
# symmetric; each m-subtile mi only computes output columns
# [0, (mi+1)*128) -- the 128-strip lower triangle -- and the host mirrors
# the rest.  Lower (not upper) so every partial PSUM write/read region
# starts at the bank base: each region then has a single writer and the
# tile framework emits single-wait copies (the walrus build rejects
# multi-wait ones).
TRI = {(0, 0), (1, 8)}

# Flat emission order of (slot_idx, mb, j, mi, w): slots in SLOTS order;
# within a slot, m-subtiles ascending -- except the LAST slot (the (1,9)
# triangle), which goes widest-first so the final computed group is the
# w=128 strip: its copy + store are the kernel tail, so make them tiny.
GROUPS = []
for _s, (_mb, _j) in enumerate(SLOTS):
    _tri = (_mb, _j) in TRI
    _mis = range(MSUB)
    if _s == NSLOT - 1:
        _mis = reversed(list(_mis))
    for _mi in _mis:
        _w = (_mi + 1) * P if _tri else B
        GROUPS.append((_s, _mb, _j, _mi, _w))
NGRP = len(GROUPS)  # 68

# store-batch boundaries over GROUP indices: after group g, store groups
# [lo, g].  At most 8 store DMAs: a store carries a data wait, so it must
# not also need a queue-reuse wait (instructions only support 1 sync wait
# in this walrus build).  Front-loaded batches; the last three stores are
# tiny (3/2/1 groups) because the SP engine serializes store transfers --
# the final stores chain back-to-back behind the last copies and gate the
# kernel tail.
_BATCH_SIZES = [18, 17, 15, 9, 5, 1, 2, 1]


def _mk_store_after(batch_sizes):
    sa = {}
    lo = 0
    for sz in batch_sizes:
        sa[lo + sz - 1] = lo
        lo += sz
    assert lo == NGRP
    return sa


STORE_AFTER = _mk_store_after(_BATCH_SIZES)

# Copies PSUM->SBUF split between DVE and ACT per store batch (whole
# batches on one engine so each store DMA still has a single-engine data
# dependency = one sync wait).  Batches 2-3 (no triangle groups -- those
# must share DVE with the garbage-column memsets) go to ACT, whose copies
# are ~2x slower; ACT also derives the stationary operand early on.
_ACT_BATCHES = {2, 3}

# Loads are split into ~128-256 KB chunks spread round-robin over the SWDGE
# queues: a single queue only sustains ~70 GB/s, so one big DMA per tensor
# would gate the first matmul ~14 us behind the st load.  j0/j1 load as
# four 128 KB quarter-chunks each (fine pacing for the head: the first
# matmuls consume them k-slice by k-slice), later blocks as 256 KB halves.


def _build_nc(repeats: int = 1, **body_kwargs):
    """Build the SPMD program.  repeats>1 runs the whole body that many
    times as sequential TileContexts (used only for timing: the delta
    between repeats isolates device time from dispatch overhead)."""
    nc = bass.Bass()
    # host-pretiled layouts: xt[j, p, o, v] = XT col-block j;
    # rel[p, o] = relation[o*P + p] (most of the stationary operand is
    # derived on device: st = rel * xt[0:2]; the first m-subtile st0 is
    # shipped from host so the first matmul needs no derive step).
    xt = nc.declare_dram_parameter(
        "xt", [NBX, P, KSUB, B], mybir.dt.bfloat16, isOutput=False
    )
    rel = nc.declare_dram_parameter(
        "rel", [P, KSUB], mybir.dt.float32, isOutput=False
    )
    # Host-shipped stationary pieces: sth[p, o, v] covers stationary
    # columns m0..m4 (v in [0, 640)): all of h=0 plus the first m-subtile
    # of h=1.  Shipping these removes the derive step from the critical
    # path of the first three slots; only m5..m7 derive on device.
    sth = nc.declare_dram_parameter(
        "sth", [P, KSUB, 5 * P], mybir.dt.bfloat16, isOutput=False
    )
    # partition-major output: out[p, g, v] = group g's row (mi*128+p) col v
    # (group g = GROUPS[g]).  Makes every store DMA a contiguous
    # per-partition blit of the staging tile; the host untangles the layout
    # during assembly.
    out = nc.declare_dram_parameter(
        "out", [P, NGRP, B], mybir.dt.float16, isOutput=True
    )
    global _SKIP_TAIL_BARRIER
    for rep in range(repeats):
        _SKIP_TAIL_BARRIER = rep == repeats - 1
        _emit_body(nc, xt, rel, sth, out, **body_kwargs)
    _SKIP_TAIL_BARRIER = False
    return nc


def _emit_body(nc, xt, rel, sth, out, do_mm=True, do_copy=True, do_store=True,
               copy_split=True, do_load=True, n_warm=5):
    store_after = STORE_AFTER
    with tile.TileContext(nc) as tc:
        with (
            tc.tile_pool(name="xt", bufs=1) as xt_pool,
            tc.tile_pool(name="st", bufs=1) as st_pool,
            tc.tile_pool(name="ob", bufs=1) as out_pool,
            tc.tile_pool(name="ps", bufs=1, space="PSUM") as psum_pool,
        ):
            rel_sb = st_pool.tile([P, KSUB], mybir.dt.float32)

            # st layout: st_sb[p, o, h*B+v] = stationary column (h*B+v) of
            # k-row (o*P+p).  The host ships m0 (first matmuls), m1-3 (rest
            # of slot 0/1's stationary) and m4 (h=1's first subtile)
            # pre-scaled; m5-7 derive on ACT below from xt block 8.  Pool
            # serializes its DMA transfers, so issue order = arrival order.
            st_sb = st_pool.tile([P, KSUB, 2 * B], mybir.dt.bfloat16)
            xt_sb = xt_pool.tile([P, NBX, KSUB, B], mybir.dt.bfloat16)
            xt_chunks = {}  # j -> list of o-starts of its chunk DMAs

            def _q(j, o0, step=1):
                nc.gpsimd.dma_start(
                    xt_sb[:, j, o0 : o0 + step, :],
                    xt[j, :, o0 : o0 + step, :],
                )

            # Interleaved head: the first stationary piece, then j0/j1 as
            # 128 KB quarter-chunks (each quarter unlocks one k-slice of
            # all four groups of the k-major slots 0-1 below -- the PE
            # consumes chunks at roughly the serialized-DMA cadence), with
            # the m1-3 stationary piece after the first moving quarter.
            # rel and m4 (first needed by slot 9, ~30us in) ride behind
            # j2; j3..j15 flow as 256 KB halves, each arriving several
            # microseconds before its slot.
            nc.gpsimd.dma_start(st_sb[:, :, 0:P], sth[:, :, 0:P])
            _q(0, 0)
            nc.gpsimd.dma_start(st_sb[:, :, P:B], sth[:, :, P:B])
            for o0 in (1, 2, 3):
                _q(0, o0)
            for o0 in range(KSUB):
                _q(1, o0)
            xt_chunks[0] = xt_chunks[1] = list(range(KSUB))
            _q(2, 0, 2)
            _q(2, 2, 2)
            xt_chunks[2] = [0, 2]
            nc.gpsimd.dma_start(rel_sb[:], rel[:])
            nc.gpsimd.dma_start(st_sb[:, :, B : B + P], sth[:, :, B : B + P])
            for j in range(3, NBX):
                if do_load:
                    _q(j, 0, 2)
                    _q(j, 2, 2)
                xt_chunks[j] = [0, 2]

            if do_mm:
                # PE warm-up source (also the ACT table-preload source):
                # zeroed SBUF, ready ~400ns in.
                warm = st_pool.tile([P, 2 * P], mybir.dt.bfloat16)
                nc.vector.memset(warm[:], 0.0)

            # ACT activation-table preload: the first activation pays a
            # ~1.3us table load; burn it at t~0.5us on a dummy copy of the
            # zeroed warm tile instead of on the rel observer at ~4us.
            act_scratch = st_pool.tile([P, 1], mybir.dt.float32)
            nc.scalar.activation(
                act_scratch[:], warm[:, 0:1],
                mybir.ActivationFunctionType.Copy,
            )

            # Tiny observer so ACT sees the rel DMA once; the stationary
            # derives below then wait only on their xt chunk.
            rel_obs = st_pool.tile([P, KSUB], mybir.dt.float32)
            nc.scalar.copy(rel_obs[:], rel_sb[:])

            # Derive the rest of the stationary operand on ACT (DVE stays
            # free for the PSUM copies): st[p, o, B+v] = rel[p, o] *
            # xt[8, p, o, v] for v in [128, 512) (m-subtiles 5-7; global
            # row-block t+8 is xt column block 8).  Each activation waits
            # on one 256 KB j8 half-chunk; j8 lands ~18us in, well before
            # the first h=1 slot (~33us).
            for o in range(KSUB):
                nc.scalar.activation(
                    st_sb[:, o, B + P : 2 * B],
                    xt_sb[:, 8, o, P:B],
                    mybir.ActivationFunctionType.Copy,
                    scale=rel_sb[:, o : o + 1],
                )

            # statically rotated PSUM banks; unique fp16 staging slot per
            # output tile (no slot reuse -> single-wait copies and stores).
            psum_big = psum_pool.tile([P, 8, B], mybir.dt.float32)
            ob_big = out_pool.tile([P, NGRP, B], mybir.dt.float16)

            if do_mm:
                # PE warm-up: dummy matmuls on the zeroed warm tile mark
                # the engine busy from t~0.5us (a cold PE runs at half
                # clock until it has been active a while).  They land in
                # bank 0's [0:128) region -- the same shape the w=128
                # triangle groups write, so the bank's region history
                # stays uniform (single-writer regions -> single-wait
                # copies).
                for _ in range(n_warm):
                    nc.tensor.matmul(
                        psum_big[:, 0, 0:P],
                        warm[:, 0:P],
                        warm[:, P : 2 * P],
                        start=True,
                        stop=True,
                    )

            # engine of each group's copy: whole store batches on one
            # engine (single-engine data dependency per store DMA).
            eng_of_grp = {}
            bi = 0
            for g in range(NGRP):
                eng_of_grp[g] = "act" if bi in _ACT_BATCHES else "dve"
                if g in store_after:
                    bi += 1

            if do_copy:
                # Zero-fill the ob columns that triangle groups never copy
                # ([w, 512)), so stores ship finite fp16 there.  One
                # rectangular memset per group; the later copies overwrite
                # the valid prefix.  On DVE (same engine as those groups'
                # copies) so stores keep a single-engine data dependency.
                for g, (s, mb, j, mi, w) in enumerate(GROUPS):
                    if w < B:
                        nc.vector.memset(ob_big[:, g, w:B], 0.0)

            # Slots 0 and 1 emit k-major (o outer, group inner): each j0/j1
            # quarter-chunk arrival unlocks one k-slice of ALL four groups
            # (~0.5-0.9us of PE work), matching the 500ns SP-load cadence,
            # instead of group-major where the first group stalls on the
            # last quarter.  Slot 0 holds banks 0-3 (triangle widths),
            # slot 1 banks 4-7 -- all eight banks live at once, legal since
            # their accumulation groups are disjoint.
            fg = MSUB       # full-group counter; slot 1 took banks 4-7
            if do_mm:
                for sl in (0, 1):
                    g0 = sl * MSUB
                    for o in range(KSUB):
                        for gi in range(MSUB):
                            s, mb, j, mi, w = GROUPS[g0 + gi]
                            bank = mi if sl == 0 else 4 + mi
                            nc.tensor.matmul(
                                psum_big[:, bank, 0:w],
                                st_sb[:, o, mi * P : (mi + 1) * P],
                                xt_sb[:, j, o, 0:w],
                                start=(o == 0),
                                stop=(o == KSUB - 1),
                            )
            if do_copy:
                for g in range(2 * MSUB):
                    s, mb, j, mi, w = GROUPS[g]
                    bank = mi if s == 0 else 4 + mi
                    nc.vector.tensor_copy(
                        ob_big[:, g, 0:w], psum_big[:, bank, 0:w]
                    )

            seen_j = {0, 1}
            cur_slot = 1
            for g, (s, mb, j, mi, w) in enumerate(GROUPS):
                if s < 2:
                    continue
                tri = (mb, j) in TRI
                if s != cur_slot:
                    cur_slot = s
                    if j not in seen_j:
                        # Dummy weight loads: make PE observe each of xt
                        # block j's chunk DMAs here (Ldweights takes one
                        # sync wait each), so the following matmuls only
                        # carry the PSUM-reuse wait.  Blocks 0 and 1 are
                        # first used by PSUM groups with no PSUM-reuse wait
                        # yet -- their matmuls absorb the chunk waits
                        # directly (one chunk per k), so no dummies there.
                        # For later blocks only the first half-chunk needs
                        # an observer: the group's k=0 matmul carries the
                        # PSUM-reuse wait, but its k=2 matmul is free to
                        # absorb the second half-chunk's wait.
                        if j >= 2:
                            nc.tensor.ldweights(
                                xt_sb[:, j, xt_chunks[j][0], 0:P]
                            )
                        seen_j.add(j)
                # triangle groups: m-subtile mi only needs columns
                # [0, (mi+1)*128) -- the host mirrors the rest.
                # PSUM banks are split by role so every bank sees a
                # constant-width write/read history (uniform regions ->
                # single-wait copies): triangle groups own banks 0-3
                # (bank = mi, width (mi+1)*128), full groups rotate over
                # banks 4-7.
                if tri:
                    bank = mi
                else:
                    bank = 4 + fg % 4
                    fg += 1
                ps = psum_big[:, bank, :]
                m0 = mb * B + mi * P
                if do_mm:
                    for k in range(KSUB):
                        nc.tensor.matmul(
                            ps[:, 0:w],
                            st_sb[:, k, m0 : m0 + P],
                            xt_sb[:, j, k, 0:w],
                            start=(k == 0),
                            stop=(k == KSUB - 1),
                        )
                if do_copy:
                    # triangle groups copy only their written columns
                    # (single-writer region -> single sync wait)
                    if copy_split and eng_of_grp[g] == "act":
                        nc.scalar.copy(ob_big[:, g, 0:w], ps[:, 0:w])
                    else:
                        nc.vector.tensor_copy(ob_big[:, g, 0:w], ps[:, 0:w])
                # Batched stores: at most 8 output DMAs total (one per HWDGE
                # queue) so no DMA ever needs both a data wait and a
                # queue-reuse wait -- instructions only support 1 sync wait.
                if do_store and g in store_after:
                    lo = store_after[g]
                    nc.sync.dma_start(
                        out[:, lo : g + 1, :],
                        ob_big[:, lo : g + 1, :],
                    )


def _make_in_maps(inputs: np.ndarray, relation: np.ndarray):
    xb = inputs.astype(ml_dtypes.bfloat16)
    # rel[p, o] = relation[o*P + p]
    rel_pd = np.ascontiguousarray(
        relation.astype(np.float32).reshape(KSUB, P).T
    )
    in_maps = []
    for t in range(C):
        # core t owns global row-blocks (t, t+8); local col-block j maps
        # to global col-block (t + j) % 16
        xr = np.roll(xb, -t * B, axis=0)          # [8192, 512]
        # xt[j, p, o, v] = xr[j*B + v, o*P + p], j = 0..15
        xt_c = np.ascontiguousarray(
            xr.reshape(NBX, B, KSUB, P).transpose(0, 3, 2, 1)
        )
        # sth[p, o, v] = rel[p, o] * x-row (v) for stationary columns
        # m0..m4 (v in [0, 640)): cols [0, 512) come from xt block 0
        # (global row-block t), col [512, 640) from block 8's first
        # subtile (global row-block t+8).  Pre-scaled on host (bf16
        # inputs, f32 multiply, bf16 round -- matches the on-device
        # derive of m5-7).
        src = np.concatenate(
            [xt_c[0], xt_c[8, :, :, 0:P]], axis=2
        ).astype(np.float32)                       # [P, KSUB, 640]
        sth_c = np.ascontiguousarray(
            rel_pd[:, :, None] * src
        ).astype(ml_dtypes.bfloat16)
        in_maps.append({"xt": xt_c, "rel": rel_pd, "sth": sth_c})
    return in_maps


def _assemble(outs: list) -> np.ndarray:
    rec = np.empty((N, N), dtype=np.float32)
    for t in range(C):
        # [128, 68, 512] partition-major; group g rows are (mi*128+p)
        o = np.asarray(outs[t], dtype=np.float32)
        for g, (s, mb, j, mi, w) in enumerate(GROUPS):
            r = (t + 8 * mb) % NB
            q = (t + j) % NB
            strip = np.ascontiguousarray(o[:, g, 0:w])   # [128, w]
            r0 = r * B + mi * P
            # valid strip rows [r0, r0+128) x cols [q*B, q*B+w)
            rec[r0 : r0 + P, q * B : q * B + w] = strip
            if q != r or (mb, j) in TRI:
                # off-diagonal blocks mirror wholesale; diagonal
                # (triangle) blocks mirror their tril strips to fill the
                # upper triangle.
                rec[q * B : q * B + w, r0 : r0 + P] = strip.T
    return rec


def kernel(inputs: np.ndarray, relation: np.ndarray) -> np.ndarray:
    nc = _build_nc()
    res = run_bass_kernel_spmd(nc, _make_in_maps(inputs, relation), list(range(C)))
    return _assemble([r["out"] for r in res.results])
